# revision 1
# baseline (speedup 1.0000x reference)
"""Trainium2 Bass kernel for a 2-layer GAT (nn_GAT_1236950581751).

Strategy (8 NeuronCores, SPMD, one program):
  - Nodes are sharded contiguously: core c owns nodes [c*12500, (c+1)*12500),
    locally reordered by in-degree (descending) so that 128-node dst tiles
    have near-uniform degree.
  - Host folds weights: layer-1 needs only h1pre = x @ (W_lin@W1) + b_lin@W1
    plus the 4 attention projections -> one [768, 68] matmul per node.
  - Device: fm = Wbig^T @ x^T (feature-major), PE-transpose to node-major,
    write a local node table [12500, 66] (h1pre + alpha_src), AllGather to the
    full table [100001, 66] (row 100000 is a dummy row used for ELL padding:
    h = 0, alpha_src = -300 so exp() underflows to 0).
  - Per dst tile (128 nodes, ELL with D_t slot-columns): one indirect DMA per
    slot column gathers table[idx[p, j]] into SBUF; DVE computes the
    segment-softmax (max-shifted exp) and the alpha-weighted sum along the
    slot axis; bias add; PE-transpose into a feature-major h1 buffer.
  - Layer 2 repeats the same structure with a [100001, 4] table
    (h2pre(3) + alpha_src2) and 1 head; output written per tile.
  - Host un-permutes the 8 output shards into the full [100000, 3] result.
"""

import numpy as np

N = 100000
C = 8                 # cores
S = N // C            # 12500 nodes per shard
P = 128
NT = (S + P - 1) // P  # 98 tiles per core
S_PAD = NT * P         # 12544
DUMMY = N              # dummy table row
TR = N + 1             # table rows
RC1 = 66               # layer-1 table row: h1pre(64) + alpha_src(2)
RC2 = 4                # layer-2 table row: h2pre(3) + alpha_src2(1)
ALPHA_PAD = -300.0
NEG_SLOPE = 0.2
H = 2                  # layer-1 heads
CH = 32                # channels per head
F1 = 68                # fm channels: 64 h1pre + 2 a_src + 2 a_dst
F2 = 5                 # fm2 channels: 3 h2pre + 1 a_src + 1 a_dst
KX = 6                 # 768 / 128 contraction chunks
NCHUNK = 512           # matmul free-dim tile

_CACHE = {}


def _fold_weights(W_lin, b_lin, W1, att_src1, att_dst1, W2, att_src2, att_dst2):
    Wf = (W_lin.astype(np.float64) @ W1.astype(np.float64))
    bf = (b_lin.astype(np.float64) @ W1.astype(np.float64))
    cols = [Wf]
    bb = [bf]
    for att in (att_src1, att_dst1):
        for h in range(H):
            a = att[h].astype(np.float64)
            cols.append((Wf[:, CH * h:CH * (h + 1)] @ a)[:, None])
            bb.append(np.array([bf[CH * h:CH * (h + 1)] @ a]))
    Wbig = np.concatenate(cols, axis=1).astype(np.float32)        # [768, 68]
    bbig = np.concatenate(bb).astype(np.float32)                  # [68]
    W2l = W2.astype(np.float64)
    P2 = np.concatenate(
        [W2l, (W2l @ att_src2[0].astype(np.float64))[:, None],
         (W2l @ att_dst2[0].astype(np.float64))[:, None]], axis=1
    ).astype(np.float32)                                          # [64, 5]
    return Wbig, bbig, P2


def _preprocess(edge_index):
    """Static graph preprocessing -> per-core idx arrays + shared schedule.

    The appended self-loop of every node is NOT put in the ELL; it is served
    on-device by an affine read of the core's own local table rows (the
    "self column"). Natural (v, v) edges in edge_index stay in the ELL.
    """
    src = np.asarray(edge_index[0], dtype=np.int64)
    dst = np.asarray(edge_index[1], dtype=np.int64)
    deg = np.bincount(dst, minlength=N).astype(np.int64)

    # CSR over dst
    order_e = np.argsort(dst, kind="stable")
    src_by_dst = src[order_e]
    rowptr = np.zeros(N + 1, np.int64)
    rowptr[1:] = np.cumsum(deg)

    # per-shard degree-descending node order
    orders = np.empty((C, S), np.int64)
    for c in range(C):
        nodes = np.arange(c * S, (c + 1) * S)
        orders[c] = nodes[np.argsort(-deg[nodes], kind="stable")]
    rank = np.empty(N, np.int64)
    for c in range(C):
        rank[orders[c]] = np.arange(S)
    shard_of = np.arange(N) // S

    # chunk-major table slot numbering (4 collective chunks, tile aligned)
    chunk_tiles = [25, 25, 25, 23]
    starts = np.array([0, 3200, 6400, 9600], np.int64)
    sizes = np.array([3200, 3200, 3200, 2900], np.int64)
    bases = np.array([0, 25600, 51200, 76800], np.int64)
    j_of = np.minimum(rank // 3200, 3)
    slot = bases[j_of] + shard_of * sizes[j_of] + (rank - starts[j_of])

    # shared per-tile max-degree schedule
    Dt = np.zeros(NT, np.int64)
    for c in range(C):
        dpad = np.zeros(S_PAD, np.int64)
        dpad[:S] = deg[orders[c]]
        Dt = np.maximum(Dt, dpad.reshape(NT, P).max(1))
    Dt = Dt.astype(np.int64)
    toff = np.zeros(NT + 1, np.int64)
    toff[1:] = np.cumsum(Dt)
    TOT = int(toff[-1])

    # per-core ELL index array [128, TOT] int32 (slot ids; DUMMY padding)
    Dmax = int(Dt.max())
    col = np.arange(Dmax)[None, :]
    idx_cores = []
    for c in range(C):
        nodes = orders[c]
        counts = deg[nodes]
        ell = np.full((S_PAD, Dmax), DUMMY, np.int32)
        mask = col < counts[:, None]
        pos = (rowptr[nodes][:, None] + col)[mask]
        rr, cc = np.nonzero(mask)
        ell[rr, cc] = slot[src_by_dst[pos]].astype(np.int32)
        idxc = np.empty((P, TOT), np.int32)
        for t in range(NT):
            idxc[:, toff[t]:toff[t + 1]] = ell[t * P:(t + 1) * P, :Dt[t]]
        idx_cores.append(idxc)

    sched = {
        "Dt": [int(d) for d in Dt],
        "toff": [int(o) for o in toff],
        "TOT": TOT,
        "chunk_tiles": chunk_tiles,
        "chunk_rows": [int(x) for x in sizes],
        "chunk_starts": [int(x) for x in starts],
        "chunk_bases": [int(x) for x in bases],
    }
    return orders, idx_cores, sched


def _build_program(sched):
    import concourse.bass as bass
    import concourse.mybir as mybir
    import concourse.tile as tile
    from concourse import bacc
    from concourse.masks import make_identity

    f32 = mybir.dt.float32
    i32 = mybir.dt.int32
    Dt = sched["Dt"]
    toff = sched["toff"]
    TOT = sched["TOT"]

    nc = bacc.Bacc("TRN2", target_bir_lowering=False, debug=False,
                   enable_asserts=False, num_devices=C)

    xT = nc.dram_tensor("xT", [768, S_PAD], f32, kind="ExternalInput")
    Wbig_d = nc.dram_tensor("Wbig", [768, F1], f32, kind="ExternalInput")
    bbig_d = nc.dram_tensor("bbig", [F1, 1], f32, kind="ExternalInput")
    P2_d = nc.dram_tensor("P2", [64, F2], f32, kind="ExternalInput")
    b1_d = nc.dram_tensor("b1", [64], f32, kind="ExternalInput")
    b2_d = nc.dram_tensor("b2", [3], f32, kind="ExternalInput")
    idx_d = nc.dram_tensor("idx", [P, TOT], i32, kind="ExternalInput")
    fp16 = mybir.dt.float16
    i16 = mybir.dt.int16
    L2TOT = sum(8 * d for d in Dt)
    SUBTOT = sum(Dt)
    G2 = 6251
    TR2 = 100016
    off2 = [0]
    soff = [0]
    for d in Dt:
        off2.append(off2[-1] + 8 * d)
        soff.append(soff[-1] + d)
    idx2_d = nc.dram_tensor("idx2", [16, L2TOT], i16, kind="ExternalInput")
    oh2_d = nc.dram_tensor("oh2", [P, 16 * SUBTOT], fp16,
                           kind="ExternalInput")
    out_d = nc.dram_tensor("out", [S, 3], f32, kind="ExternalOutput")

    # local tables split per collective chunk so each AllGather only waits
    # for its own chunk's rows
    tb1_locs = [nc.dram_tensor(f"tb1_loc{j}", [sched["chunk_rows"][j], RC1], f32,
                               kind="Internal") for j in range(4)]
    tb1_full = nc.dram_tensor("tb1_full", [TR, RC1], f32, kind="Internal",
                              addr_space="Shared")
    tb2_locs = [nc.dram_tensor(f"tb2_loc{j}", [sched["chunk_rows"][j], 8], fp16,
                               kind="Internal") for j in range(4)]
    tb2_full = nc.dram_tensor("tb2_full", [TR2, 8], fp16, kind="Internal",
                              addr_space="Shared")
    cstarts = sched["chunk_starts"]

    def loc_write(locs, row0, rows, src_ap):
        j = min(row0 // 3200, 3)
        nc.sync.dma_start(out=locs[j][row0 - cstarts[j]:row0 - cstarts[j] + rows, :],
                          in_=src_ap)

    # matmul N chunks
    chunks = []
    c0 = 0
    while c0 < S_PAD:
        cw = min(NCHUNK, S_PAD - c0)
        chunks.append((c0, cw))
        c0 += cw

    from contextlib import ExitStack

    with tile.TileContext(nc) as tc, ExitStack() as stack:
        const = stack.enter_context(tc.tile_pool(name="const", bufs=1))
        big = stack.enter_context(tc.tile_pool(name="big", bufs=1))
        io = stack.enter_context(tc.tile_pool(name="io", bufs=3))
        fmp = stack.enter_context(tc.tile_pool(name="fmp", bufs=2))
        nmp = stack.enter_context(tc.tile_pool(name="nmp", bufs=3))
        gp = stack.enter_context(tc.tile_pool(name="gp", bufs=2))
        ixp = stack.enter_context(tc.tile_pool(name="ixp", bufs=4))
        l2p = stack.enter_context(tc.tile_pool(name="l2p", bufs=3))
        wk = stack.enter_context(tc.tile_pool(name="wk", bufs=4))
        ps = stack.enter_context(tc.tile_pool(name="ps", bufs=2, space="PSUM"))

        # ---- constants ----
        wtiles = const.tile([P, KX, F1], f32)
        for k in range(KX):
            nc.sync.dma_start(out=wtiles[:, k, :], in_=Wbig_d[k * P:(k + 1) * P, :])
        bbig_sb = const.tile([F1, 1], f32)
        nc.sync.dma_start(out=bbig_sb[:], in_=bbig_d[:])
        p2_sb = const.tile([64, F2], f32)
        nc.sync.dma_start(out=p2_sb[:], in_=P2_d[:])
        b1_bc = const.tile([P, 64], f32)
        nc.sync.dma_start(out=b1_bc[:], in_=bass.AP(
            tensor=b1_d, offset=0, ap=[[0, P], [1, 64]]))
        b2_bc = const.tile([P, 3], f32)
        nc.sync.dma_start(out=b2_bc[:], in_=bass.AP(
            tensor=b2_d, offset=0, ap=[[0, P], [1, 3]]))
        id68 = const.tile([F1, F1], f32)
        make_identity(nc, id68[:])
        id128 = const.tile([P, P], f32)
        make_identity(nc, id128[:])
        id5 = const.tile([F2, F2], f32)
        make_identity(nc, id5[:])
        ebias = const.tile([P, 1], f32)
        nc.vector.memset(ebias[:], -8.0)

        idx_all = big.tile([P, TOT], i32)
        nc.sync.dma_start(out=idx_all[:], in_=idx_d[:])
        h1T_all = big.tile([64, S_PAD], f32)
        aD1 = big.tile([P, 2 * NT], f32)
        aD2 = big.tile([P, NT], f32)

        # dummy rows
        dummy1 = const.tile([1, RC1], f32)
        nc.vector.memset(dummy1[:], 0.0)
        nc.vector.memset(dummy1[:, 64:66], ALPHA_PAD)
        nc.sync.dma_start(out=tb1_full[DUMMY:DUMMY + 1, :], in_=dummy1[:])
        dummy2 = const.tile([16, 8], fp16)
        nc.vector.memset(dummy2[:], 0.0)
        nc.vector.memset(dummy2[0:1, 3:4], ALPHA_PAD)
        nc.sync.dma_start(out=tb2_full[TR2 - 16:TR2, :], in_=dummy2[:])

        # ---- STEP A: fm = Wbig^T @ x^T, transpose, write local table ----
        scopeA = nc.named_scope("stepA"); scopeA.__enter__()
        t_idx = 0
        for (cst, cw) in chunks:
            ps_fm = ps.tile([F1, cw], f32, tag="fm")
            for k in range(KX):
                xt = io.tile([P, cw], f32, tag="xt")
                nc.sync.dma_start(out=xt[:], in_=xT[k * P:(k + 1) * P, cst:cst + cw])
                nc.tensor.matmul(out=ps_fm[:], lhsT=wtiles[:, k, :], rhs=xt[:],
                                 start=(k == 0), stop=(k == KX - 1))
            fm_sb = fmp.tile([F1, cw], f32, tag="fm_sb")
            nc.vector.tensor_scalar(out=fm_sb[:], in0=ps_fm[:],
                                    scalar1=bbig_sb[:, 0:1], scalar2=None,
                                    op0=mybir.AluOpType.add)
            for sub in range(cw // P):
                t = t_idx
                t_idx += 1
                ps_tr = ps.tile([P, F1], f32, tag="tr")
                nc.tensor.transpose(out=ps_tr[:], in_=fm_sb[:, sub * P:(sub + 1) * P],
                                    identity=id68[:])
                nm = nmp.tile([P, F1], f32, tag="nm")
                nc.vector.tensor_copy(out=nm[:], in_=ps_tr[:])
                rows = min(P, S - t * P)
                if rows > 0:
                    loc_write(tb1_locs, t * P, rows, nm[:rows, 0:RC1])
                nc.vector.tensor_copy(out=aD1[:, 2 * t:2 * t + 2], in_=nm[:, 66:68])

        # ---- STEP B: chunked AllGather of table1 ----
        scopeA.__exit__(None, None, None)
        scopeB = nc.named_scope("ag1"); scopeB.__enter__()
        for j in range(4):
            sz = sched["chunk_rows"][j]
            bs = sched["chunk_bases"][j]
            nc.gpsimd.collective_compute(
                "AllGather", mybir.AluOpType.bypass,
                replica_groups=[list(range(C))],
                ins=[tb1_locs[j][:].opt()],
                outs=[tb1_full[bs:bs + C * sz, :].opt()],
            )

        # ---- STEP C: layer-1 aggregation per tile ----
        scopeB.__exit__(None, None, None)
        def gat_tile(t, tbl, tbl_locs, rc, nch, aD, out_cb):
            """Gather + segment softmax + weighted sum for dst tile t.

            Slots [0, D) are gathered via indirect DMA; slot D (the appended
            self-loop) is an affine read of this core's local table rows.
            """
            D = Dt[t]
            DG = D + 1
            g = gp.tile([P, DG * rc], f32, tag=f"g{rc}",
                        padded_shape=[P, (max(Dt) + 1) * rc])
            for j in range(D):
                nc.gpsimd.indirect_dma_start(
                    out=g[:, j * rc:(j + 1) * rc],
                    out_offset=None,
                    in_=tbl[:],
                    in_offset=bass.IndirectOffsetOnAxis(
                        ap=idx_all[:, toff[t] + j:toff[t] + j + 1], axis=0),
                )
            jch = min((t * P) // 3200, 3)
            lrow = t * P - cstarts[jch]
            srows = min(P, S - t * P)
            nc.sync.dma_start(out=g[:srows, D * rc:DG * rc],
                              in_=tbl_locs[jch][lrow:lrow + srows, :])
            g3 = g[:].rearrange("p (d c) -> p d c", c=rc)
            if nch == 64:  # layer 1, H=2 heads
                asv = g3[:, :, 64:66].transpose([0, 2, 1])        # [P,2,DG]
                adv = aD[:, 2 * t:2 * t + 2].unsqueeze(2).to_broadcast([P, 2, DG])
                e = wk.tile([P, 2, DG], f32, tag="e")
                nc.vector.tensor_tensor(out=e[:], in0=asv, in1=adv,
                                        op=mybir.AluOpType.add)
                tmp = wk.tile([P, 2, DG], f32, tag="tmp")
                nc.vector.tensor_scalar_mul(tmp[:], e[:], NEG_SLOPE)
                nc.vector.tensor_tensor(out=e[:], in0=e[:], in1=tmp[:],
                                        op=mybir.AluOpType.max)
                negm = wk.tile([P, 2], f32, tag="negm")
                nc.vector.tensor_reduce(out=negm[:], in_=e[:],
                                        axis=mybir.AxisListType.X,
                                        op=mybir.AluOpType.max, negate=True)
                pp = wk.tile([P, 2, DG], f32, tag="pp")
                for h in range(2):
                    nc.scalar.activation(out=pp[:, h, :], in_=e[:, h, :],
                                         func=mybir.ActivationFunctionType.Exp,
                                         bias=negm[:, h:h + 1], scale=1.0)
                den = wk.tile([P, 2], f32, tag="den")
                nc.vector.tensor_reduce(out=den[:], in_=pp[:],
                                        axis=mybir.AxisListType.X,
                                        op=mybir.AluOpType.add)
                inv = wk.tile([P, 2], f32, tag="inv")
                nc.vector.reciprocal(inv[:], den[:])
                hv = g3[:, :, 0:64].rearrange("p d (h c) -> p d h c", h=2)
                pv = pp[:].transpose([0, 2, 1]).unsqueeze(3).to_broadcast(
                    [P, DG, 2, CH])
                nc.vector.tensor_tensor(out=hv, in0=hv, in1=pv,
                                        op=mybir.AluOpType.mult)
                o = wk.tile([P, 2, CH], f32, tag="o")
                nc.vector.tensor_reduce(out=o[:], in_=hv.transpose([0, 2, 3, 1]),
                                        axis=mybir.AxisListType.X,
                                        op=mybir.AluOpType.add)
                invv = inv[:].unsqueeze(2).to_broadcast([P, 2, CH])
                nc.vector.tensor_tensor(out=o[:], in0=o[:], in1=invv,
                                        op=mybir.AluOpType.mult)
                h1 = wk.tile([P, 64], f32, tag="h1")
                nc.vector.tensor_tensor(out=h1[:], in0=o[:].rearrange("p h c -> p (h c)"),
                                        in1=b1_bc[:], op=mybir.AluOpType.add)
                out_cb(t, h1)
            else:  # layer 2, 1 head, 3 channels
                as2 = g3[:, :, 3:4].squeeze(2)                    # [P, DG]
                e = wk.tile([P, DG], f32, tag="e2")
                nc.vector.tensor_scalar(out=e[:], in0=as2, scalar1=aD[:, t:t + 1],
                                        scalar2=None, op0=mybir.AluOpType.add)
                tmp = wk.tile([P, DG], f32, tag="tmp2")
                nc.vector.tensor_scalar_mul(tmp[:], e[:], NEG_SLOPE)
                nc.vector.tensor_tensor(out=e[:], in0=e[:], in1=tmp[:],
                                        op=mybir.AluOpType.max)
                negm = wk.tile([P, 1], f32, tag="negm2")
                nc.vector.tensor_reduce(out=negm[:], in_=e[:],
                                        axis=mybir.AxisListType.X,
                                        op=mybir.AluOpType.max, negate=True)
                pp = wk.tile([P, DG], f32, tag="pp2")
                nc.scalar.activation(out=pp[:], in_=e[:],
                                     func=mybir.ActivationFunctionType.Exp,
                                     bias=negm[:, 0:1], scale=1.0)
                den = wk.tile([P, 1], f32, tag="den2")
                nc.vector.tensor_reduce(out=den[:], in_=pp[:],
                                        axis=mybir.AxisListType.X,
                                        op=mybir.AluOpType.add)
                inv = wk.tile([P, 1], f32, tag="inv2")
                nc.vector.reciprocal(inv[:], den[:])
                hv = g3[:, :, 0:3]
                pv = pp[:].unsqueeze(2).to_broadcast([P, DG, 3])
                nc.vector.tensor_tensor(out=hv, in0=hv, in1=pv,
                                        op=mybir.AluOpType.mult)
                o = wk.tile([P, 3], f32, tag="o2")
                nc.vector.tensor_reduce(out=o[:], in_=hv.transpose([0, 2, 1]),
                                        axis=mybir.AxisListType.X,
                                        op=mybir.AluOpType.add)
                nc.vector.tensor_scalar(out=o[:], in0=o[:], scalar1=inv[:, 0:1],
                                        scalar2=None, op0=mybir.AluOpType.mult)
                nc.vector.tensor_tensor(out=o[:], in0=o[:], in1=b2_bc[:],
                                        op=mybir.AluOpType.add)
                out_cb(t, o)

        def l1_out(t, h1):
            ps_h1t = ps.tile([64, P], f32, tag="h1t")
            nc.tensor.transpose(out=ps_h1t[:], in_=h1[:], identity=id128[:])
            nc.vector.tensor_copy(out=h1T_all[:, t * P:(t + 1) * P], in_=ps_h1t[:])

        scopeC = nc.named_scope("layer1"); scopeC.__enter__()
        for t in range(NT):
            gat_tile(t, tb1_full, tb1_locs, RC1, 64, aD1, l1_out)
        scopeC.__exit__(None, None, None)

        # ---- STEP D: layer-2 node projections + table2 ----
        scopeD = nc.named_scope("stepD"); scopeD.__enter__()
        t_idx = 0
        for (cst, cw) in chunks:
            ps2 = ps.tile([F2, cw], f32, tag="fm")
            nc.tensor.matmul(out=ps2[:], lhsT=p2_sb[:], rhs=h1T_all[:, cst:cst + cw],
                             start=True, stop=True)
            fm2 = fmp.tile([F2, cw], f32, tag="fm2_sb")
            nc.vector.tensor_copy(out=fm2[:], in_=ps2[:])
            for sub in range(cw // P):
                t = t_idx
                t_idx += 1
                ps_tr2 = ps.tile([P, F2], f32, tag="tr")
                nc.tensor.transpose(out=ps_tr2[:], in_=fm2[:, sub * P:(sub + 1) * P],
                                    identity=id5[:])
                nm2 = nmp.tile([P, 8], fp16, tag="nm2")
                nc.vector.memset(nm2[:], 0.0)
                nc.vector.tensor_copy(out=nm2[:, 0:4], in_=ps_tr2[:, 0:4])
                rows = min(P, S - t * P)
                if rows > 0:
                    loc_write(tb2_locs, t * P, rows, nm2[:rows, :])
                nc.vector.tensor_copy(out=aD2[:, t:t + 1], in_=ps_tr2[:, 4:5])

        scopeD.__exit__(None, None, None)
        scopeG = nc.named_scope("ag2"); scopeG.__enter__()
        for j in range(4):
            sz = sched["chunk_rows"][j]
            bs = sched["chunk_bases"][j]
            nc.gpsimd.collective_compute(
                "AllGather", mybir.AluOpType.bypass,
                replica_groups=[list(range(C))],
                ins=[tb2_locs[j][:].opt()],
                outs=[tb2_full[bs:bs + C * sz, :].opt()],
            )
        scopeG.__exit__(None, None, None)

        # ---- STEP E: layer-2 aggregation ----
        def l2_out(t, o):
            rows = min(P, S - t * P)
            if rows > 0:
                nc.sync.dma_start(out=out_d[t * P:t * P + rows, :], in_=o[:rows, :])

        DMAX = max(Dt)

        def l2_tile(t):
            DT = Dt[t]
            Lw = off2[t + 1] - off2[t]
            ixt = ixp.tile([P, Lw], i16, tag="ix2",
                           padded_shape=[P, 8 * DMAX])
            nc.sync.dma_start(out=ixt[:], in_=bass.AP(
                tensor=idx2_d, offset=off2[t],
                ap=[[0, 8], [L2TOT, 16], [1, Lw]]))
            g2 = l2p.tile([P, DT, 128], fp16, tag="g2",
                          padded_shape=[P, DMAX, 128])
            ni = P * DT
            nc.gpsimd.dma_gather(
                g2[:, 0:DT, :],
                bass.AP(tensor=tb2_full, offset=0, ap=[[128, G2], [1, 128]]),
                ixt[:, 0:ni // 16],
                ni, ni, 128, single_packet=False)
            oh = l2p.tile([P, DT, 16], fp16, tag="oh",
                          padded_shape=[P, DMAX, 16])
            nc.sync.dma_start(out=oh[:], in_=oh2_d[:, 16 * soff[t]:
                                                   16 * soff[t + 1]])
            g2v = g2[:, 0:DT, :].rearrange("p d (s w) -> p d s w", s=16)
            ov = oh[:].unsqueeze(3).to_broadcast([P, DT, 16, 8])
            nc.vector.tensor_tensor(out=g2v, in0=g2v, in1=ov,
                                    op=mybir.AluOpType.mult)
            s2 = l2p.tile([P, DT + 1, 8], fp16, tag="s2",
                          padded_shape=[P, DMAX + 1, 8])
            with nc.allow_low_precision(reason="one-hot select: 15 of 16 "
                                        "summands are exact zeros"):
                nc.vector.tensor_reduce(out=s2[:, 0:DT, :],
                                        in_=g2v.transpose([0, 1, 3, 2]),
                                        axis=mybir.AxisListType.X,
                                        op=mybir.AluOpType.add)
            jch = min((t * P) // 3200, 3)
            lrow = t * P - sched["chunk_starts"][jch]
            srows = min(P, S - t * P)
            nc.sync.dma_start(out=s2[:srows, DT, :],
                              in_=tb2_locs[jch][lrow:lrow + srows, :])
            DG = DT + 1
            e = wk.tile([P, DG], fp16, tag="e2", padded_shape=[P, DMAX + 1])
            nc.vector.tensor_scalar(out=e[:], in0=s2[:, :, 3],
                                    scalar1=aD2[:, t:t + 1], scalar2=None,
                                    op0=mybir.AluOpType.add)
            tmp = wk.tile([P, DG], fp16, tag="tmp2",
                          padded_shape=[P, DMAX + 1])
            nc.vector.tensor_scalar_mul(tmp[:], e[:], NEG_SLOPE)
            nc.vector.tensor_tensor(out=e[:], in0=e[:], in1=tmp[:],
                                    op=mybir.AluOpType.max)
            pp = wk.tile([P, DG], fp16, tag="pp2", padded_shape=[P, DMAX + 1])
            nc.scalar.activation(out=pp[:], in_=e[:],
                                 func=mybir.ActivationFunctionType.Exp,
                                 bias=ebias[:, 0:1], scale=1.0)
            den = wk.tile([P, 1], f32, tag="den2")
            nc.vector.tensor_reduce(out=den[:], in_=pp[:],
                                    axis=mybir.AxisListType.X,
                                    op=mybir.AluOpType.add)
            nc.vector.tensor_scalar(out=den[:], in0=den[:], scalar1=1e-6,
                                    scalar2=None, op0=mybir.AluOpType.add)
            inv = wk.tile([P, 1], f32, tag="inv2")
            nc.vector.reciprocal(inv[:], den[:])
            hv = s2[:, :, 0:3]
            pv = pp[:].unsqueeze(2).to_broadcast([P, DG, 3])
            nc.vector.tensor_tensor(out=hv, in0=hv, in1=pv,
                                    op=mybir.AluOpType.mult)
            o = wk.tile([P, 3], f32, tag="o2")
            nc.vector.tensor_reduce(out=o[:], in_=hv.transpose([0, 2, 1]),
                                    axis=mybir.AxisListType.X,
                                    op=mybir.AluOpType.add)
            nc.vector.tensor_scalar(out=o[:], in0=o[:], scalar1=inv[:, 0:1],
                                    scalar2=None, op0=mybir.AluOpType.mult)
            nc.vector.tensor_tensor(out=o[:], in0=o[:], in1=b2_bc[:],
                                    op=mybir.AluOpType.add)
            l2_out(t, o)

        scopeE = nc.named_scope("layer2"); scopeE.__enter__()
        for t in range(NT):
            l2_tile(t)
        scopeE.__exit__(None, None, None)

    nc.compile()
    return nc


def _prepare(inputs):
    x = np.asarray(inputs["x"], dtype=np.float32)
    edge_index = np.asarray(inputs["edge_index"])
    orders, idx_cores, sched = _preprocess(edge_index)
    Wbig, bbig, P2 = _fold_weights(
        np.asarray(inputs["W_lin"], np.float32), np.asarray(inputs["b_lin"], np.float32),
        np.asarray(inputs["W1"], np.float32), np.asarray(inputs["att_src1"], np.float32),
        np.asarray(inputs["att_dst1"], np.float32), np.asarray(inputs["W2"], np.float32),
        np.asarray(inputs["att_src2"], np.float32), np.asarray(inputs["att_dst2"], np.float32))
    b1 = np.asarray(inputs["b1"], np.float32)
    b2 = np.asarray(inputs["b2"], np.float32)

    toff = sched["toff"]
    in_maps = []
    for c in range(C):
        xs = np.zeros((768, S_PAD), np.float32)
        xs[:, :S] = x[orders[c]].T
        idxc = idx_cores[c].astype(np.int64)
        blocks2, subs = [], []
        eye16 = np.eye(16, dtype=np.float16)
        for t in range(NT):
            ell2 = idxc[:, toff[t]:toff[t + 1]]           # [P, Dt] slot ids
            flat = ell2.T.reshape(-1)
            blocks2.append((flat >> 4).astype(np.int16).reshape(-1, 16).T)
            subs.append(eye16[ell2 & 15].reshape(P, -1))  # [P, Dt*16] one-hot
        in_maps.append({
            "xT": np.ascontiguousarray(xs),
            "Wbig": Wbig, "bbig": bbig[:, None].copy(), "P2": P2,
            "b1": b1, "b2": b2,
            "idx": idx_cores[c],
            "idx2": np.ascontiguousarray(np.concatenate(blocks2, axis=1)),
            "oh2": np.ascontiguousarray(np.concatenate(subs, axis=1)),
        })
    return orders, sched, in_maps


def kernel(**inputs):
    import time
    from concourse.bass_utils import run_bass_kernel_spmd

    orders, sched, in_maps = _prepare(inputs)
    key = ("prog", tuple(sched["Dt"]))
    if key not in _CACHE:
        _CACHE[key] = _build_program(sched)
    nc = _CACHE[key]

    res = None
    for attempt in range(3):
        try:
            res = run_bass_kernel_spmd(nc, in_maps, core_ids=list(range(C)),
                                       trace=False)
            break
        except Exception:
            # transient NRT_EXEC_UNIT_UNRECOVERABLE wedges recover after ~60s
            if attempt == 2:
                raise
            time.sleep(75)
    out = np.empty((N, 3), np.float32)
    for c in range(C):
        out[orders[c]] = res.results[c]["out"]
    return out


# ---------------------------------------------------------------------------
# numpy golden model of the device pipeline (for test harnesses)
def golden(**inputs):
    x = np.asarray(inputs["x"], np.float32)
    orders, idx_cores, sched = _preprocess(np.asarray(inputs["edge_index"]))
    Wbig, bbig, P2 = _fold_weights(
        np.asarray(inputs["W_lin"], np.float32), np.asarray(inputs["b_lin"], np.float32),
        np.asarray(inputs["W1"], np.float32), np.asarray(inputs["att_src1"], np.float32),
        np.asarray(inputs["att_dst1"], np.float32), np.asarray(inputs["W2"], np.float32),
        np.asarray(inputs["att_src2"], np.float32), np.asarray(inputs["att_dst2"], np.float32))
    b1 = np.asarray(inputs["b1"], np.float32)
    b2 = np.asarray(inputs["b2"], np.float32)
    Dt = sched["Dt"]
    toff = sched["toff"]

    # tables
    tb1 = np.zeros((TR, RC1), np.float32)
    tb2 = np.zeros((TR, RC2), np.float32)
    tb1[DUMMY, 64:66] = ALPHA_PAD
    tb2[DUMMY, 3] = ALPHA_PAD

    fms = []
    for c in range(C):
        fm = (x[orders[c]] @ Wbig + bbig).astype(np.float32)      # [S, 68]
        fms.append(fm)

    # slot mapping identical to _preprocess
    rank = np.arange(S)
    j_of = np.minimum(rank // 3200, 3)
    sizes = np.array(sched["chunk_rows"]); starts = np.array(sched["chunk_starts"])
    bases = np.array(sched["chunk_bases"])
    for c in range(C):
        slots = bases[j_of] + c * sizes[j_of] + (rank - starts[j_of])
        tb1[slots] = fms[c][:, 0:RC1]

    def leaky(v):
        return np.where(v >= 0, v, NEG_SLOPE * v)

    out = np.empty((N, 3), np.float32)
    h1T = {}
    for c in range(C):
        idxc = idx_cores[c]
        aD1 = np.zeros((S_PAD, 2), np.float32)
        aD1[:S] = fms[c][:, 66:68]
        selfrows1 = np.zeros((S_PAD, RC1), np.float32)
        selfrows1[:S] = fms[c][:, 0:RC1]
        h1c = np.zeros((S_PAD, 64), np.float32)
        for t in range(NT):
            gg = tb1[idxc[:, toff[t]:toff[t + 1]]]                 # [128, D, 66]
            g = np.concatenate([gg, selfrows1[t * P:(t + 1) * P][:, None, :]], 1)
            asv = g[:, :, 64:66]
            adv = aD1[t * P:(t + 1) * P][:, None, :]
            e = leaky(asv + adv)
            m = e.max(axis=1, keepdims=True)
            pp = np.exp(e - m)
            den = pp.sum(axis=1, keepdims=True)
            inv = 1.0 / den
            hh = g[:, :, 0:64].reshape(P, -1, 2, CH)
            o = (hh * pp[:, :, :, None]).sum(axis=1) * inv[:, 0, :, None]
            h1c[t * P:(t + 1) * P] = o.reshape(P, 64) + b1
        h1T[c] = h1c
        fm2 = h1c[:S] @ P2                                         # [S, 5]
        slots = bases[j_of] + c * sizes[j_of] + (rank - starts[j_of])
        tb2[slots] = fm2[:, 0:RC2]
        h1T[c] = (h1c, fm2)

    for c in range(C):
        idxc = idx_cores[c]
        h1c, fm2 = h1T[c]
        aD2 = np.zeros((S_PAD,), np.float32)
        aD2[:S] = fm2[:, 4]
        selfrows2 = np.zeros((S_PAD, RC2), np.float32)
        selfrows2[:S] = fm2[:, 0:RC2]
        oc = np.zeros((S, 3), np.float32)
        for t in range(NT):
            gg = tb2[idxc[:, toff[t]:toff[t + 1]]]                 # [128, D, 4]
            g = np.concatenate([gg, selfrows2[t * P:(t + 1) * P][:, None, :]], 1)
            e = leaky(g[:, :, 3] + aD2[t * P:(t + 1) * P][:, None])
            m = e.max(axis=1, keepdims=True)
            pp = np.exp(e - m)
            den = pp.sum(axis=1, keepdims=True)
            o = (g[:, :, 0:3] * pp[:, :, None]).sum(axis=1) / den
            rows = min(P, S - t * P)
            oc[t * P:t * P + rows] = o[:rows] + b2
        out[orders[c]] = oc
    return out



# revision 10
# speedup vs baseline: 1.5335x; 1.5335x over previous
"""Trainium2 Bass kernel for a 2-layer GAT (nn_GAT_1236950581751).

v2 strategy (8 NeuronCores, SPMD, one program):
  - Nodes sharded contiguously, locally reordered by in-degree (desc).
  - Host folds weights: one [768, 68] matmul per node gives h1pre(64) +
    asrc1(2) + adst1(2) projections.
  - stepA: fm = Wbig^T @ x^T in fp16, PE-transpose, write fp16 node table
    rows (h only, 128B) to 4 local chunk tables; AllGather to the full
    [100000, 64] fp16 table (= 25000 packets of 512B / 4 rows).
  - Layer-1 aggregation per dst tile (128 nodes, shared ELL of Dt slots):
    the tile's slots are split into 4 column-quarters; each quarter is ONE
    dma_gather of 512B packets on its own SWDGE queue (4-queue concurrency
    doubles random-gather throughput; packet ids fit int16 since
    packet = row>>2 <= 25000).  Row selection within each packet is a
    host-shipped one-hot (zero for ELL padding) applied as a multiply +
    contiguous halving tree (no strided tensor_scalar -- pathological).
    asrc is recomputed on device as <h_sel, a_src> per head.  Softmax and
    the weighted sum run node-major as before; the self loop is served by
    an affine read of the local chunk table + e computed in stepA.
  - stepD: [64, 5] matmul -> fp16 [100000, 8] table2 (= 6250 packets of
    256B / 16 rows), AllGather.
  - Layer-2 aggregation: same structure with 256B packets, 16-row one-hot
    (folded with the softmax weights), halving tree, 4 queues.
"""

import numpy as np

N = 100000
C = 8                 # cores
S = N // C            # 12500 nodes per shard
P = 128
NT = (S + P - 1) // P  # 98 tiles per core
S_PAD = NT * P         # 12544
NEG_SLOPE = 0.2
H = 2                  # layer-1 heads
CH = 32                # channels per head
F1 = 68                # fm channels: 64 h1pre + 2 a_src + 2 a_dst
F2 = 5                 # fm2 channels: 3 h2pre + 1 a_src + 1 a_dst
KX = 6                 # 768 / 128 contraction chunks
NCHUNK = 512           # matmul free-dim tile
PK1 = N // 4           # 25000 layer-1 packets (512B = 4 rows x 128B)
PK2 = N // 16          # 6250 layer-2 packets (256B = 16 rows x 16B)
EMASK = -300.0

_CACHE = {}


def _fold_weights(W_lin, b_lin, W1, att_src1, att_dst1, W2, att_src2, att_dst2):
    Wf = (W_lin.astype(np.float64) @ W1.astype(np.float64))
    bf = (b_lin.astype(np.float64) @ W1.astype(np.float64))
    cols = [Wf]
    bb = [bf]
    for att in (att_src1, att_dst1):
        for h in range(H):
            a = att[h].astype(np.float64)
            cols.append((Wf[:, CH * h:CH * (h + 1)] @ a)[:, None])
            bb.append(np.array([bf[CH * h:CH * (h + 1)] @ a]))
    Wbig = np.concatenate(cols, axis=1).astype(np.float32)        # [768, 68]
    bbig = np.concatenate(bb).astype(np.float32)                  # [68]
    W2l = W2.astype(np.float64)
    P2 = np.concatenate(
        [W2l, (W2l @ att_src2[0].astype(np.float64))[:, None],
         (W2l @ att_dst2[0].astype(np.float64))[:, None]], axis=1
    ).astype(np.float32)                                          # [64, 5]
    return Wbig, bbig, P2


def _preprocess(edge_index):
    """Static graph preprocessing -> per-core packed idx/onehot arrays."""
    src = np.asarray(edge_index[0], dtype=np.int64)
    dst = np.asarray(edge_index[1], dtype=np.int64)
    deg = np.bincount(dst, minlength=N).astype(np.int64)

    order_e = np.argsort(dst, kind="stable")
    src_by_dst = src[order_e]
    rowptr = np.zeros(N + 1, np.int64)
    rowptr[1:] = np.cumsum(deg)

    orders = np.empty((C, S), np.int64)
    for c in range(C):
        nodes = np.arange(c * S, (c + 1) * S)
        orders[c] = nodes[np.argsort(-deg[nodes], kind="stable")]
    rank = np.empty(N, np.int64)
    for c in range(C):
        rank[orders[c]] = np.arange(S)
    shard_of = np.arange(N) // S

    # chunk-major table slot numbering (4 collective chunks, tile aligned)
    chunk_tiles = [25, 25, 25, 23]
    starts = np.array([0, 3200, 6400, 9600], np.int64)
    sizes = np.array([3200, 3200, 3200, 2900], np.int64)
    bases = np.array([0, 25600, 51200, 76800], np.int64)
    j_of = np.minimum(rank // 3200, 3)
    slot = bases[j_of] + shard_of * sizes[j_of] + (rank - starts[j_of])

    # shared per-tile max-degree schedule
    Dt = np.zeros(NT, np.int64)
    for c in range(C):
        dpad = np.zeros(S_PAD, np.int64)
        dpad[:S] = deg[orders[c]]
        Dt = np.maximum(Dt, dpad.reshape(NT, P).max(1))
    Dt = np.maximum(Dt.astype(np.int64), 4)  # >=4 so quarters are nonempty
    toff = np.zeros(NT + 1, np.int64)
    toff[1:] = np.cumsum(Dt)
    TOT = int(toff[-1])

    # per-core ELL slot array [S_PAD rows, per-tile Dt cols]; -1 = padding
    Dmax = int(Dt.max())
    col = np.arange(Dmax)[None, :]
    ells = []
    for c in range(C):
        nodes = orders[c]
        counts = deg[nodes]
        ell = np.full((S_PAD, Dmax), -1, np.int64)
        mask = col < counts[:, None]
        pos = (rowptr[nodes][:, None] + col)[mask]
        rr, cc = np.nonzero(mask)
        ell[rr, cc] = slot[src_by_dst[pos]]
        ells.append(ell)

    sched = {
        "Dt": [int(d) for d in Dt],
        "toff": [int(o) for o in toff],
        "TOT": TOT,
        "chunk_tiles": chunk_tiles,
        "chunk_rows": [int(x) for x in sizes],
        "chunk_starts": [int(x) for x in starts],
        "chunk_bases": [int(x) for x in bases],
    }
    return orders, ells, sched


def _quarters(D):
    """Split D columns into 4 contiguous ranges (sizes differ by <=1)."""
    q = D // 4
    r = D % 4
    sizes = [q + (1 if i < r else 0) for i in range(4)]
    offs = [0]
    for s_ in sizes:
        offs.append(offs[-1] + s_)
    return sizes, offs


def _pack_streams(ells, Dt, toff):
    """Build per-core packed gather/one-hot streams.

    Returns per-core dicts with:
      idx1 [16, 8*TOT] i16  : layer-1 packet ids (row>>2), i = d*128+p order
      idx2 [16, 8*TOT] i16  : layer-2 packet ids (row>>4)
      oh4  [P, 4*TOT]  f16  : layer-1 one-hot over sub4 (0 for padding)
      oh2  [P, 16*TOT] f16  : layer-2 one-hot over sub16 (0 for padding)
      em   [P, TOT]    f16  : 0 for real slots, -300 for padding
    """
    NTt = len(Dt)
    out = []
    eye4 = np.eye(4, dtype=np.float16)
    eye2 = np.eye(16, dtype=np.float16)
    for c in range(C):
        ell = ells[c]
        i1_blocks, i2_blocks, o4_blocks, o2_blocks, em_blocks = [], [], [], [], []
        for t in range(NTt):
            D = Dt[t]
            e = ell[t * P:(t + 1) * P, :D]               # [P, D] slot or -1
            valid = e >= 0
            er = np.where(valid, e, 0)
            flat = er.T.reshape(-1)                       # i = d*128+p
            i1_blocks.append((flat >> 2).astype(np.int16).reshape(-1, 16).T)
            i2_blocks.append((flat >> 4).astype(np.int16).reshape(-1, 16).T)
            o4 = eye4[er & 3] * valid[:, :, None]         # [P, D, 4]
            o2 = eye2[er & 15] * valid[:, :, None]        # [P, D, 16]
            o4_blocks.append(o4.reshape(P, -1).astype(np.float16))
            o2_blocks.append(o2.reshape(P, -1).astype(np.float16))
            em_blocks.append(np.where(valid, 0, EMASK).astype(np.float16))
        out.append({
            "idx1": np.ascontiguousarray(np.concatenate(i1_blocks, axis=1)),
            "idx2": np.ascontiguousarray(np.concatenate(i2_blocks, axis=1)),
            "oh4": np.ascontiguousarray(np.concatenate(o4_blocks, axis=1)),
            "oh2": np.ascontiguousarray(np.concatenate(o2_blocks, axis=1)),
            "em": np.ascontiguousarray(np.concatenate(em_blocks, axis=1)),
        })
    return out


def _build_program(sched):
    import concourse.bass as bass
    import concourse.mybir as mybir
    import concourse.tile as tile
    from concourse import bacc
    from concourse.masks import make_identity
    from contextlib import ExitStack

    f32 = mybir.dt.float32
    fp16 = mybir.dt.float16
    i16 = mybir.dt.int16
    Dt = sched["Dt"]
    toff = sched["toff"]
    TOT = sched["TOT"]
    DMAX = max(Dt)
    QMAX = (DMAX + 3) // 4
    cstarts = sched["chunk_starts"]
    csizes = sched["chunk_rows"]
    cbases = sched["chunk_bases"]

    nc = bacc.Bacc("TRN2", target_bir_lowering=False, debug=False,
                   enable_asserts=False, num_devices=C, num_swdge_queues=4)

    xT = nc.dram_tensor("xT", [768, S_PAD], fp16, kind="ExternalInput")
    Wbig_d = nc.dram_tensor("Wbig", [768, F1], fp16, kind="ExternalInput")
    bbig_d = nc.dram_tensor("bbig", [F1, 1], f32, kind="ExternalInput")
    P2_d = nc.dram_tensor("P2", [64, F2], f32, kind="ExternalInput")
    b1_d = nc.dram_tensor("b1", [64], f32, kind="ExternalInput")
    b2_d = nc.dram_tensor("b2", [3], f32, kind="ExternalInput")
    asrc1_d = nc.dram_tensor("asrc1", [1, 64], fp16, kind="ExternalInput")
    idx1_d = nc.dram_tensor("idx1", [16, 8 * TOT], i16, kind="ExternalInput")
    idx2_d = nc.dram_tensor("idx2", [16, 8 * TOT], i16, kind="ExternalInput")
    oh4_d = nc.dram_tensor("oh4", [P, 4 * TOT], fp16, kind="ExternalInput")
    oh2_d = nc.dram_tensor("oh2", [P, 16 * TOT], fp16, kind="ExternalInput")
    em_d = nc.dram_tensor("em", [P, TOT], fp16, kind="ExternalInput")
    out_d = nc.dram_tensor("out", [S, 3], f32, kind="ExternalOutput")

    tb1_locs = [nc.dram_tensor(f"tb1_loc{j}", [csizes[j], 64], fp16,
                               kind="Internal") for j in range(4)]
    tb1_full = nc.dram_tensor("tb1_full", [N, 64], fp16, kind="Internal",
                              addr_space="Shared")
    tb2_locs = [nc.dram_tensor(f"tb2_loc{j}", [csizes[j], 8], fp16,
                               kind="Internal") for j in range(4)]
    tb2_full = nc.dram_tensor("tb2_full", [N, 8], fp16, kind="Internal",
                              addr_space="Shared")

    chunks = []
    c0 = 0
    while c0 < S_PAD:
        cw = min(NCHUNK, S_PAD - c0)
        chunks.append((c0, cw))
        c0 += cw

    with tile.TileContext(nc) as tc, ExitStack() as stack:
        const = stack.enter_context(tc.tile_pool(name="const", bufs=1))
        big = stack.enter_context(tc.tile_pool(name="big", bufs=1))
        io = stack.enter_context(tc.tile_pool(name="io", bufs=3))
        fmp = stack.enter_context(tc.tile_pool(name="fmp", bufs=2))
        nmp = stack.enter_context(tc.tile_pool(name="nmp", bufs=3))
        gp = stack.enter_context(tc.tile_pool(name="gp", bufs=6))
        ixp = stack.enter_context(tc.tile_pool(name="ixp", bufs=3))
        trp = stack.enter_context(tc.tile_pool(name="trp", bufs=2))
        wk = stack.enter_context(tc.tile_pool(name="wk", bufs=3))
        ps = stack.enter_context(tc.tile_pool(name="ps", bufs=2, space="PSUM"))

        # ---- constants ----
        wtiles = const.tile([P, KX, F1], fp16)
        for k in range(KX):
            nc.sync.dma_start(out=wtiles[:, k, :], in_=Wbig_d[k * P:(k + 1) * P, :])
        bbig_sb = const.tile([F1, 1], f32)
        nc.sync.dma_start(out=bbig_sb[:], in_=bbig_d[:])
        p2_sb = const.tile([64, F2], f32)
        nc.sync.dma_start(out=p2_sb[:], in_=P2_d[:])
        b1_bc = const.tile([P, 64], f32)
        nc.sync.dma_start(out=b1_bc[:], in_=bass.AP(
            tensor=b1_d, offset=0, ap=[[0, P], [1, 64]]))
        b2_bc = const.tile([P, 3], f32)
        nc.sync.dma_start(out=b2_bc[:], in_=bass.AP(
            tensor=b2_d, offset=0, ap=[[0, P], [1, 3]]))
        asrc_bc = const.tile([P, 64], fp16)
        nc.sync.dma_start(out=asrc_bc[:], in_=bass.AP(
            tensor=asrc1_d, offset=0, ap=[[0, P], [1, 64]]))
        id68 = const.tile([F1, F1], f32)
        make_identity(nc, id68[:])
        id128 = const.tile([P, P], f32)
        make_identity(nc, id128[:])
        id5 = const.tile([F2, F2], f32)
        make_identity(nc, id5[:])

        h1T_all = big.tile([64, S_PAD], f32)
        aD1 = big.tile([P, 2 * NT], f32)
        aD2 = big.tile([P, NT], f32)
        eS1 = big.tile([P, 2 * NT], f32)
        eS2 = big.tile([P, NT], f32)

        def loc_write(locs, row0, rows, src_ap):
            j = min(row0 // 3200, 3)
            nc.sync.dma_start(
                out=locs[j][row0 - cstarts[j]:row0 - cstarts[j] + rows, :],
                in_=src_ap)

        # ---- STEP A ----
        scopeA = nc.named_scope("stepA"); scopeA.__enter__()
        t_idx = 0
        for (cst, cw) in chunks:
            ps_fm = ps.tile([F1, cw], f32, tag="fm")
            for k in range(KX):
                xt = io.tile([P, cw], fp16, tag="xt")
                nc.sync.dma_start(out=xt[:], in_=xT[k * P:(k + 1) * P, cst:cst + cw])
                nc.tensor.matmul(out=ps_fm[:], lhsT=wtiles[:, k, :], rhs=xt[:],
                                 start=(k == 0), stop=(k == KX - 1))
            fm_sb = fmp.tile([F1, cw], f32, tag="fm_sb")
            nc.vector.tensor_scalar(out=fm_sb[:], in0=ps_fm[:],
                                    scalar1=bbig_sb[:, 0:1], scalar2=None,
                                    op0=mybir.AluOpType.add)
            for sub in range(cw // P):
                t = t_idx
                t_idx += 1
                ps_tr = ps.tile([P, F1], f32, tag="tr")
                nc.tensor.transpose(out=ps_tr[:], in_=fm_sb[:, sub * P:(sub + 1) * P],
                                    identity=id68[:])
                nm = nmp.tile([P, F1], f32, tag="nm")
                nc.vector.tensor_copy(out=nm[:], in_=ps_tr[:])
                nm16 = nmp.tile([P, 64], fp16, tag="nm16")
                nc.vector.tensor_copy(out=nm16[:], in_=nm[:, 0:64])
                rows = min(P, S - t * P)
                if rows > 0:
                    loc_write(tb1_locs, t * P, rows, nm16[:rows, :])
                nc.vector.tensor_copy(out=aD1[:, 2 * t:2 * t + 2], in_=nm[:, 66:68])
                nc.vector.tensor_tensor(out=eS1[:, 2 * t:2 * t + 2],
                                        in0=nm[:, 64:66], in1=nm[:, 66:68],
                                        op=mybir.AluOpType.add)

        scopeA.__exit__(None, None, None)
        scopeB = nc.named_scope("ag1"); scopeB.__enter__()
        for j in range(4):
            sz = csizes[j]
            bs = cbases[j]
            nc.gpsimd.collective_compute(
                "AllGather", mybir.AluOpType.bypass,
                replica_groups=[list(range(C))],
                ins=[tb1_locs[j][:].opt()],
                outs=[tb1_full[bs:bs + C * sz, :].opt()],
            )
        scopeB.__exit__(None, None, None)

        # ---- layer-1 aggregation ----
        def l1_tile(t):
            D = Dt[t]
            DG = D + 1
            qs, qo = _quarters(D)
            hsel = trp.tile([P, DG, 64], fp16, tag="hsel",
                            padded_shape=[P, DMAX + 1, 64])
            oh4t = ixp.tile([P, D, 4], fp16, tag="oh4",
                            padded_shape=[P, DMAX, 4])
            nc.sync.dma_start(out=oh4t[:], in_=oh4_d[:, 4 * toff[t]:
                                                     4 * toff[t] + 4 * D])
            emt = ixp.tile([P, D], fp16, tag="em", padded_shape=[P, DMAX])
            nc.sync.dma_start(out=emt[:], in_=em_d[:, toff[t]:toff[t] + D])
            for q in range(4):
                qlen = qs[q]
                if qlen == 0:
                    continue
                ni = P * qlen
                ix = ixp.tile([P, 8 * qlen], i16, tag=f"ix1_{q}",
                              padded_shape=[P, 8 * QMAX])
                nc.sync.dma_start(out=ix[:], in_=bass.AP(
                    tensor=idx1_d, offset=8 * (toff[t] + qo[q]),
                    ap=[[0, 8], [8 * TOT, 16], [1, 8 * qlen]]))
                g4 = gp.tile([P, qlen, 256], fp16, tag="g4",
                             padded_shape=[P, QMAX, 256])
                nc.gpsimd.dma_gather(
                    g4[:], bass.AP(tensor=tb1_full, offset=0,
                                   ap=[[256, PK1], [1, 256]]),
                    ix[:], ni, ni, 256, single_packet=False, queue_num=q)
                # masked select: multiply by one-hot, in-place halving tree
                ohq = oh4t[:, qo[q]:qo[q] + qlen, :]
                ohb = ohq.unsqueeze(3).to_broadcast([P, qlen, 4, 64])
                g4v = g4[:].rearrange("p d (k c) -> p d k c", k=4)
                nc.vector.tensor_tensor(out=g4v, in0=g4v, in1=ohb,
                                        op=mybir.AluOpType.mult)
                nc.vector.tensor_tensor(out=g4[:, :, 0:128],
                                        in0=g4[:, :, 0:128], in1=g4[:, :, 128:256],
                                        op=mybir.AluOpType.add)
                nc.vector.tensor_tensor(out=hsel[:, qo[q]:qo[q] + qlen, :],
                                        in0=g4[:, :, 0:64], in1=g4[:, :, 64:128],
                                        op=mybir.AluOpType.add)
            # self row
            jch = min((t * P) // 3200, 3)
            lrow = t * P - cstarts[jch]
            srows = min(P, S - t * P)
            nc.sync.dma_start(out=hsel[:srows, D, :],
                              in_=tb1_locs[jch][lrow:lrow + srows, :])
            # asrc per head: <h_sel, a_src>
            e = wk.tile([P, 2, DG], fp16, tag="e", padded_shape=[P, 2, DMAX + 1])
            for h in range(2):
                hah = wk.tile([P, D, CH], fp16, tag="hah",
                              padded_shape=[P, DMAX, CH])
                abh = asrc_bc[:, CH * h:CH * (h + 1)].unsqueeze(1) \
                    .to_broadcast([P, D, CH])
                nc.vector.tensor_tensor(out=hah[:],
                                        in0=hsel[:, 0:D, CH * h:CH * (h + 1)],
                                        in1=abh, op=mybir.AluOpType.mult)
                asr = wk.tile([P, D], fp16, tag="asr", padded_shape=[P, DMAX])
                with nc.allow_low_precision(reason="32-term fp16 dot, values ~0.1"):
                    nc.vector.tensor_reduce(out=asr[:], in_=hah[:],
                                            axis=mybir.AxisListType.X,
                                            op=mybir.AluOpType.add)
                nc.vector.tensor_tensor(out=asr[:], in0=asr[:], in1=emt[:],
                                        op=mybir.AluOpType.add)
                nc.vector.tensor_scalar(out=e[:, h, 0:D], in0=asr[:],
                                        scalar1=aD1[:, 2 * t + h:2 * t + h + 1],
                                        scalar2=None, op0=mybir.AluOpType.add)
            nc.vector.tensor_copy(out=e[:, :, D:DG],
                                  in_=eS1[:, 2 * t:2 * t + 2].unsqueeze(2))
            tmp = wk.tile([P, 2, DG], fp16, tag="tmp", padded_shape=[P, 2, DMAX + 1])
            nc.vector.tensor_scalar_mul(tmp[:], e[:], NEG_SLOPE)
            nc.vector.tensor_tensor(out=e[:], in0=e[:], in1=tmp[:],
                                    op=mybir.AluOpType.max)
            negm = wk.tile([P, 2], f32, tag="negm")
            nc.vector.tensor_reduce(out=negm[:], in_=e[:],
                                    axis=mybir.AxisListType.X,
                                    op=mybir.AluOpType.max, negate=True)
            pp = wk.tile([P, 2, DG], fp16, tag="pp", padded_shape=[P, 2, DMAX + 1])
            for h in range(2):
                nc.scalar.activation(out=pp[:, h, :], in_=e[:, h, :],
                                     func=mybir.ActivationFunctionType.Exp,
                                     bias=negm[:, h:h + 1], scale=1.0)
            den = wk.tile([P, 2], f32, tag="den")
            nc.vector.tensor_reduce(out=den[:], in_=pp[:],
                                    axis=mybir.AxisListType.X,
                                    op=mybir.AluOpType.add)
            inv = wk.tile([P, 2], f32, tag="inv")
            nc.vector.reciprocal(inv[:], den[:])
            # alpha-weight per channel group (in place on hsel)
            for h in range(2):
                ppb = pp[:, h, :].unsqueeze(2).to_broadcast([P, DG, CH])
                nc.vector.tensor_tensor(out=hsel[:, :, CH * h:CH * (h + 1)],
                                        in0=hsel[:, :, CH * h:CH * (h + 1)],
                                        in1=ppb, op=mybir.AluOpType.mult)
            o = wk.tile([P, 64], f32, tag="o")
            nc.vector.tensor_reduce(out=o[:], in_=hsel[:].transpose([0, 2, 1]),
                                    axis=mybir.AxisListType.X,
                                    op=mybir.AluOpType.add)
            for h in range(2):
                nc.vector.tensor_scalar(out=o[:, CH * h:CH * (h + 1)],
                                        in0=o[:, CH * h:CH * (h + 1)],
                                        scalar1=inv[:, h:h + 1], scalar2=None,
                                        op0=mybir.AluOpType.mult)
            nc.vector.tensor_tensor(out=o[:], in0=o[:], in1=b1_bc[:],
                                    op=mybir.AluOpType.add)
            ps_h1t = ps.tile([64, P], f32, tag="h1t")
            nc.tensor.transpose(out=ps_h1t[:], in_=o[:], identity=id128[:])
            nc.vector.tensor_copy(out=h1T_all[:, t * P:(t + 1) * P], in_=ps_h1t[:])

        scopeC = nc.named_scope("layer1"); scopeC.__enter__()
        for t in range(NT):
            l1_tile(t)
        scopeC.__exit__(None, None, None)

        # ---- stepD: layer-2 node projections + table2 ----
        scopeD = nc.named_scope("stepD"); scopeD.__enter__()
        t_idx = 0
        for (cst, cw) in chunks:
            ps2 = ps.tile([F2, cw], f32, tag="fm")
            nc.tensor.matmul(out=ps2[:], lhsT=p2_sb[:], rhs=h1T_all[:, cst:cst + cw],
                             start=True, stop=True)
            fm2 = fmp.tile([F2, cw], f32, tag="fm2_sb")
            nc.vector.tensor_copy(out=fm2[:], in_=ps2[:])
            for sub in range(cw // P):
                t = t_idx
                t_idx += 1
                ps_tr2 = ps.tile([P, F2], f32, tag="tr")
                nc.tensor.transpose(out=ps_tr2[:], in_=fm2[:, sub * P:(sub + 1) * P],
                                    identity=id5[:])
                nm2f = nmp.tile([P, F2], f32, tag="nm2f")
                nc.vector.tensor_copy(out=nm2f[:], in_=ps_tr2[:])
                nm2 = nmp.tile([P, 8], fp16, tag="nm2")
                nc.vector.memset(nm2[:], 0.0)
                nc.vector.tensor_copy(out=nm2[:, 0:4], in_=nm2f[:, 0:4])
                rows = min(P, S - t * P)
                if rows > 0:
                    loc_write(tb2_locs, t * P, rows, nm2[:rows, :])
                nc.vector.tensor_copy(out=aD2[:, t:t + 1], in_=nm2f[:, 4:5])
                nc.vector.tensor_tensor(out=eS2[:, t:t + 1],
                                        in0=nm2f[:, 3:4], in1=nm2f[:, 4:5],
                                        op=mybir.AluOpType.add)

        scopeD.__exit__(None, None, None)
        scopeG = nc.named_scope("ag2"); scopeG.__enter__()
        for j in range(4):
            sz = csizes[j]
            bs = cbases[j]
            nc.gpsimd.collective_compute(
                "AllGather", mybir.AluOpType.bypass,
                replica_groups=[list(range(C))],
                ins=[tb2_locs[j][:].opt()],
                outs=[tb2_full[bs:bs + C * sz, :].opt()],
            )
        scopeG.__exit__(None, None, None)

        # ---- layer-2 aggregation ----
        def l2_tile(t):
            D = Dt[t]
            DG = D + 1
            qs, qo = _quarters(D)
            oh2t = ixp.tile([P, D, 16], fp16, tag="oh2",
                            padded_shape=[P, DMAX, 16])
            nc.sync.dma_start(out=oh2t[:], in_=oh2_d[:, 16 * toff[t]:
                                                     16 * toff[t] + 16 * D])
            emt = ixp.tile([P, D], fp16, tag="em2", padded_shape=[P, DMAX])
            nc.sync.dma_start(out=emt[:], in_=em_d[:, toff[t]:toff[t] + D])
            g2 = trp.tile([P, D, 128], fp16, tag="g2",
                          padded_shape=[P, DMAX, 128])
            for q in range(4):
                qlen = qs[q]
                if qlen == 0:
                    continue
                ni = P * qlen
                ix = ixp.tile([P, 8 * qlen], i16, tag=f"ix2_{q}",
                              padded_shape=[P, 8 * QMAX])
                nc.sync.dma_start(out=ix[:], in_=bass.AP(
                    tensor=idx2_d, offset=8 * (toff[t] + qo[q]),
                    ap=[[0, 8], [8 * TOT, 16], [1, 8 * qlen]]))
                nc.gpsimd.dma_gather(
                    g2[:, qo[q]:qo[q] + qlen, :],
                    bass.AP(tensor=tb2_full, offset=0, ap=[[128, PK2], [1, 128]]),
                    ix[:], ni, ni, 128, single_packet=False, queue_num=q)
            # as2 = one-hot dot of channel 3
            g2v4 = g2[:].rearrange("p d (r c) -> p d r c", r=16)
            v3 = g2v4[:, :, :, 3:4].squeeze(3)
            tmp16 = wk.tile([P, D, 16], fp16, tag="tmp16",
                            padded_shape=[P, DMAX, 16])
            nc.vector.tensor_tensor(out=tmp16[:], in0=v3, in1=oh2t[:],
                                    op=mybir.AluOpType.mult)
            as2 = wk.tile([P, D], fp16, tag="as2", padded_shape=[P, DMAX])
            with nc.allow_low_precision(reason="one-hot select: 15 of 16 "
                                        "summands are exact zeros"):
                nc.vector.tensor_reduce(out=as2[:], in_=tmp16[:],
                                        axis=mybir.AxisListType.X,
                                        op=mybir.AluOpType.add)
            nc.vector.tensor_tensor(out=as2[:], in0=as2[:], in1=emt[:],
                                    op=mybir.AluOpType.add)
            e = wk.tile([P, DG], fp16, tag="e2", padded_shape=[P, DMAX + 1])
            nc.vector.tensor_scalar(out=e[:, 0:D], in0=as2[:],
                                    scalar1=aD2[:, t:t + 1], scalar2=None,
                                    op0=mybir.AluOpType.add)
            nc.vector.tensor_copy(out=e[:, D:DG], in_=eS2[:, t:t + 1])
            tmp = wk.tile([P, DG], fp16, tag="tmp2", padded_shape=[P, DMAX + 1])
            nc.vector.tensor_scalar_mul(tmp[:], e[:], NEG_SLOPE)
            nc.vector.tensor_tensor(out=e[:], in0=e[:], in1=tmp[:],
                                    op=mybir.AluOpType.max)
            negm = wk.tile([P, 1], f32, tag="negm2")
            nc.vector.tensor_reduce(out=negm[:], in_=e[:],
                                    axis=mybir.AxisListType.X,
                                    op=mybir.AluOpType.max, negate=True)
            pp = wk.tile([P, DG], fp16, tag="pp2", padded_shape=[P, DMAX + 1])
            nc.scalar.activation(out=pp[:], in_=e[:],
                                 func=mybir.ActivationFunctionType.Exp,
                                 bias=negm[:, 0:1], scale=1.0)
            den = wk.tile([P, 1], f32, tag="den2")
            nc.vector.tensor_reduce(out=den[:], in_=pp[:],
                                    axis=mybir.AxisListType.X,
                                    op=mybir.AluOpType.add)
            nc.vector.tensor_scalar(out=den[:], in0=den[:], scalar1=1e-6,
                                    scalar2=None, op0=mybir.AluOpType.add)
            inv = wk.tile([P, 1], f32, tag="inv2")
            nc.vector.reciprocal(inv[:], den[:])
            # fold alpha into the one-hot, multiply, tree
            ppb = pp[:, 0:D].unsqueeze(2).to_broadcast([P, D, 16])
            nc.vector.tensor_tensor(out=oh2t[:], in0=oh2t[:], in1=ppb,
                                    op=mybir.AluOpType.mult)
            g2v = g2[:].rearrange("p d (r c) -> p d r c", r=16)
            ohb = oh2t[:].unsqueeze(3).to_broadcast([P, D, 16, 8])
            nc.vector.tensor_tensor(out=g2v, in0=g2v, in1=ohb,
                                    op=mybir.AluOpType.mult)
            nc.vector.tensor_tensor(out=g2[:, :, 0:64], in0=g2[:, :, 0:64],
                                    in1=g2[:, :, 64:128], op=mybir.AluOpType.add)
            nc.vector.tensor_tensor(out=g2[:, :, 0:32], in0=g2[:, :, 0:32],
                                    in1=g2[:, :, 32:64], op=mybir.AluOpType.add)
            nc.vector.tensor_tensor(out=g2[:, :, 0:16], in0=g2[:, :, 0:16],
                                    in1=g2[:, :, 16:32], op=mybir.AluOpType.add)
            s8 = wk.tile([P, DG, 8], fp16, tag="s8", padded_shape=[P, DMAX + 1, 8])
            nc.vector.tensor_tensor(out=s8[:, 0:D, :], in0=g2[:, :, 0:8],
                                    in1=g2[:, :, 8:16], op=mybir.AluOpType.add)
            jch = min((t * P) // 3200, 3)
            lrow = t * P - cstarts[jch]
            srows = min(P, S - t * P)
            nc.sync.dma_start(out=s8[:srows, D, :],
                              in_=tb2_locs[jch][lrow:lrow + srows, :])
            ppself = pp[:, D:DG].unsqueeze(2).to_broadcast([P, 1, 8])
            nc.vector.tensor_tensor(out=s8[:, D:DG, :], in0=s8[:, D:DG, :],
                                    in1=ppself, op=mybir.AluOpType.mult)
            o = wk.tile([P, 8], f32, tag="o2")
            nc.vector.tensor_reduce(out=o[:], in_=s8[:].transpose([0, 2, 1]),
                                    axis=mybir.AxisListType.X,
                                    op=mybir.AluOpType.add)
            nc.vector.tensor_scalar(out=o[:, 0:3], in0=o[:, 0:3],
                                    scalar1=inv[:, 0:1], scalar2=None,
                                    op0=mybir.AluOpType.mult)
            nc.vector.tensor_tensor(out=o[:, 0:3], in0=o[:, 0:3], in1=b2_bc[:],
                                    op=mybir.AluOpType.add)
            rows = min(P, S - t * P)
            if rows > 0:
                nc.sync.dma_start(out=out_d[t * P:t * P + rows, :],
                                  in_=o[:rows, 0:3])

        scopeE = nc.named_scope("layer2"); scopeE.__enter__()
        for t in range(NT):
            l2_tile(t)
        scopeE.__exit__(None, None, None)

    nc.compile()
    return nc


def _prepare(inputs):
    x = np.asarray(inputs["x"], dtype=np.float32)
    edge_index = np.asarray(inputs["edge_index"])
    orders, ells, sched = _preprocess(edge_index)
    Wbig, bbig, P2 = _fold_weights(
        np.asarray(inputs["W_lin"], np.float32), np.asarray(inputs["b_lin"], np.float32),
        np.asarray(inputs["W1"], np.float32), np.asarray(inputs["att_src1"], np.float32),
        np.asarray(inputs["att_dst1"], np.float32), np.asarray(inputs["W2"], np.float32),
        np.asarray(inputs["att_src2"], np.float32), np.asarray(inputs["att_dst2"], np.float32))
    b1 = np.asarray(inputs["b1"], np.float32)
    b2 = np.asarray(inputs["b2"], np.float32)
    asrc1 = np.asarray(inputs["att_src1"], np.float16).reshape(1, 64)

    streams = _pack_streams(ells, sched["Dt"], sched["toff"])
    in_maps = []
    for c in range(C):
        xs = np.zeros((768, S_PAD), np.float16)
        xs[:, :S] = x[orders[c]].T.astype(np.float16)
        m = {
            "xT": np.ascontiguousarray(xs),
            "Wbig": Wbig.astype(np.float16), "bbig": bbig[:, None].copy(),
            "P2": P2, "b1": b1, "b2": b2, "asrc1": asrc1,
        }
        m.update(streams[c])
        in_maps.append(m)
    return orders, sched, in_maps


def kernel(**inputs):
    import time
    from concourse.bass_utils import run_bass_kernel_spmd

    orders, sched, in_maps = _prepare(inputs)
    key = ("prog", tuple(sched["Dt"]))
    if key not in _CACHE:
        _CACHE[key] = _build_program(sched)
    nc = _CACHE[key]

    res = None
    for attempt in range(3):
        try:
            res = run_bass_kernel_spmd(nc, in_maps, core_ids=list(range(C)),
                                       trace=False)
            break
        except Exception:
            if attempt == 2:
                raise
            time.sleep(75)
    out = np.empty((N, 3), np.float32)
    for c in range(C):
        out[orders[c]] = res.results[c]["out"]
    return out
